# revision 35
# baseline (speedup 1.0000x reference)
"""Multi-head causal attention (B=2, S=2048, E=1024, H=16, D=64) on 8 TRN2
NeuronCores.

Sharding: 4-way tensor-parallel over heads x 2-way data-parallel over batch.
Core c handles batch b = c//4 and head group g = c%4 (heads 4g..4g+3).

Per-core device program:
  - Q/K projections run entirely in fp8e4 with MatmulPerfMode.DoubleRow
    (two 128-deep contraction tiles per instruction): lhsT = fp8(32*Wq/Wk)
    et-pair slices, rhs = fp8(X^T) et-pair slices, fp32 PSUM. The 32x
    weight prescale keeps W (std 0.02) out of the fp8 subnormal range; the
    resulting 1024x score scale is folded into the exp.
  - V = X @ Wv stays bf16 (fp8 there pushes max-rel-err past the 2e-2
    gate); a ones column is appended per head (V_aug) so the softmax
    denominators fall out of the PV matmul for free.
  - Q/K PSUM results are stored as fp8e4 tiles with a zero second
    DoubleRow slot, so the score matmuls also run DoubleRow at half cost:
    ST[k, q] = lhsT kt8[64, 2, 128] x rhs qt8[64, 2, q]. The two heads of
    a pair write one [P, 2, QC] two-bank PSUM tile and a single wide exp
    (ScalarE, PSUM -> SBUF bf16, scale = 1/(8*1024)) covers both heads.
  - Causal masking is post-exp: Pool-engine affine_select zeroes the lower
    triangle of the diagonal 128-wide strip of stx (no PE mask matmuls);
    fully-masked columns are skipped via the narrow-diag trick.
  - ctxT[d, q] (+ sums row) = V_aug^T @ P^T accumulated over k tiles in
    PSUM (bf16). PE is in-order, so the emission order is the schedule:
    there is no separate projection phase — Q/K/V projection groups for
    chunk qc+1 and out-proj matmuls for chunk qc-1 are laced one item per
    unit into chunk qc's attention stream (sharing the po PSUM ring), so
    the first exp fires ~6-9us in and projection work soaks up PE slack
    while ScalarE grinds through the exps (the ACT engine is the pacing
    engine for the attention phase). PV for unit ki is emitted pv_depth
    units late so its exp/select have score-matmul time to finish.
  - Normalize: ctx (+sums) -> SBUF f32 copies free the PV banks at once;
    1/sums (f32r) is broadcast across partitions by a K=1 f32r matmul
    (no DMA round-trip) and multiplied on DVE -> ctxn bf16. The whole
    chain is deferred into the next head-pair's first units so it never
    stalls the PE stream.
  - outT partial [e, q] = Wo_shard^T @ ctxn per q chunk; bf16 partials,
    two e-tiles per output DMA.
  - Input DMAs are fused per qc chunk (one descriptor batch per chunk)
    and double-buffered, so a chained iteration's fill overlaps the
    previous iteration's compute and the first projection starts ~3us in.

Host side: gather 8 partial outT tensors, out[b] = sum_g partial^T + bo.
"""

import numpy as np
import ml_dtypes

import bass_rust
import concourse.bass as bass
import concourse.tile as tile
from concourse import mybir
from concourse.tile import TileContext
from concourse.bass_utils import run_bass_kernel_spmd
from concourse.vector_clock import ScopedClock

# ---------------------------------------------------------------------------
# Workaround: this environment's walrus build caps the number of sync-wait
# commands encodable on a single instruction. Redistribute excess waits onto
# single-wait NOPs inserted immediately before the instruction on the same
# engine queue (program order preserves the blocking semantics).
# ---------------------------------------------------------------------------


def _patched_drain_and_barrier(self, tick_clock, wait_clock):
    nop0 = self.nc.sync.nop(nofuse=True)
    wait_clock.add_sem_waits(nop0.ins, ScopedClock({None: tick_clock.global_clock}))
    si = nop0.ins.sync_info
    if si is not None and si.on_wait and len(si.on_wait) > 1:
        waits = list(si.on_wait)
        nop0.ins.sync_info = bass_rust.SyncInfo(
            on_wait=[waits[0]], on_update=list(si.on_update or [])
        )
        for w in waits[1:]:
            n = self.nc.sync.nop(nofuse=True)
            n.ins.sync_info = bass_rust.SyncInfo(on_wait=[w], on_update=[])
    self.nc.sync.drain()
    self.nc.all_engine_barrier()
    assert self.sems is not None
    popped = self.nc._tile_sem_poison_stack.pop()
    assert popped is self._sem_poison
    self.nc.clear_and_free_semaphores(list(self.sems.allocated().values()))
    self.nc.all_engine_barrier()


tile.TileContext._drain_and_barrier = _patched_drain_and_barrier

MAX_WAITS = 1


def split_excess_waits(nc, max_waits=None):
    if max_waits is None:
        max_waits = MAX_WAITS
    for f in nc.m.functions:
        for bb in f.blocks:
            insts = bb.instructions
            out = []
            changed = False
            for inst in insts:
                si = inst.sync_info
                waits = list(si.on_wait) if si is not None and si.on_wait else []
                if len(waits) > max_waits:
                    changed = True
                    excess, keep = waits[:-max_waits], waits[-max_waits:]
                    for w in excess:
                        out.append(mybir.InstNoOp(
                            name=nc.get_next_instruction_name(),
                            engine=inst.engine,
                            bass_nofuse=True,
                            sync_info=mybir.SyncInfo(on_wait=[w], on_update=[]),
                        ))
                    inst.sync_info = mybir.SyncInfo(
                        on_wait=keep, on_update=list(si.on_update or [])
                    )
                out.append(inst)
            if changed:
                bb.instructions = out


# ---------------------------------------------------------------------------
# Problem constants (hardcoded per the harness contract).
# ---------------------------------------------------------------------------

B, S, E = 2, 2048, 1024
H, D = 16, 64
NCORES, TP = 8, 4
HPC = H // TP          # heads per core = 4
DH = HPC * D           # head-dim columns per core = 256
P = 128                # partitions
QC = 512               # q chunk (free dim of score matmuls)
NQC = S // QC          # 4 q chunks
NKT = S // P           # 16 k tiles
SCALE = 1.0 / np.sqrt(np.float32(D))

OPTS = {
    "st_bufs": 2,
    "stx_bufs": 7,
    "po_bufs": 2,
    "pv_depth": 4,
    "narrow": True,
}

BF = mybir.dt.bfloat16
F32 = mybir.dt.float32
FP8 = mybir.dt.float8e4
Exp = mybir.ActivationFunctionType.Exp
MULT = mybir.AluOpType.mult
DR = mybir.MatmulPerfMode.DoubleRow


class _Persist:
    pass


def _declare_io(nc):
    io = _Persist()
    # inputs are pre-arranged on host to [partition, chunk, free] so every
    # DMA is contiguous per partition
    io.xt = nc.dram_tensor("xt", [P, E // P, S], BF, kind="ExternalInput").ap()
    io.xt8 = nc.dram_tensor("xt8", [P, E // P, S], FP8, kind="ExternalInput").ap()
    io.wq8 = nc.dram_tensor("wq8", [P, E // P, DH], FP8, kind="ExternalInput").ap()
    io.wk8 = nc.dram_tensor("wk8", [P, E // P, DH], FP8, kind="ExternalInput").ap()
    io.wv = nc.dram_tensor("wv", [P, E // P, DH], BF, kind="ExternalInput").ap()
    io.wo = nc.dram_tensor("wo", [P, DH // P, E], BF, kind="ExternalInput").ap()
    io.outp = nc.dram_tensor("outp", [E, S], BF, kind="ExternalOutput").ap()
    return io


ET = E // P   # 8 e tiles
NM = QC // P  # 4 k-tiles per q chunk
NDT = DH // P  # 2 head-pair column blocks


def _persistent(ctx, tc):
    nc = tc.nc
    ps = _Persist()
    consts = ctx.enter_context(tc.tile_pool(name="consts", bufs=1))

    # xt + weights are double-buffered so a chained iteration's input fill
    # overlaps the previous iteration's compute
    ps.in_pool = ctx.enter_context(tc.tile_pool(name="inp", bufs=2))
    # fp8 Q/K with a zero second DoubleRow slot:
    # qt8[qc]: [P, pair, slot, q]; kt8[ki]: [P, pair, slot, k]
    ps.qt8_q = [consts.tile([P, NDT, 2, QC], FP8, tag=f"qt{qc}",
                            name=f"qt{qc}") for qc in range(NQC)]
    ps.kt8_q = [consts.tile([P, NDT, 2, QC], FP8, tag=f"kt{kq}", name=f"kt{kq}")
                for kq in range(NQC)]
    ps.v_t = [consts.tile([P, HPC, 66], BF, tag=f"v{ki}", name=f"v{ki}")
              for ki in range(NKT)]
    ps.ctxn_q = [consts.tile([P, DH // P, QC], BF, tag=f"ctxn{qc}",
                             name=f"ctxn{qc}") for qc in range(NQC)]

    # f32r ones column for the linv PE-broadcast matmul (memset cannot
    # target f32r, so fill f32 scratch and convert with a DVE copy)
    ps.ones_col = consts.tile([1, 64], mybir.dt.float32r, tag="ones64")
    ones_f = consts.tile([1, 64], F32, tag="ones64f")
    nc.gpsimd.memset(ones_f[:], 1.0)
    with nc.allow_low_precision(reason="f32->f32r ones for linv bcast"):
        nc.vector.tensor_copy(out=ps.ones_col[:], in_=ones_f[:])

    # zero DoubleRow slots (written once; iterations only touch slot 0)
    for qc in range(NQC):
        nc.gpsimd.memset(ps.qt8_q[qc][:, :, 1, :], 0.0)
    for kq in range(NQC):
        nc.gpsimd.memset(ps.kt8_q[kq][:, :, 1, :], 0.0)
    for ki in range(NKT):
        # ones column for V_aug
        nc.gpsimd.memset(ps.v_t[ki][:, :, 64:66], 0.0)
        nc.gpsimd.memset(ps.v_t[ki][:, :, 64:65], 1.0)

    ps.stx_pool = ctx.enter_context(tc.tile_pool(name="stx", bufs=OPTS["stx_bufs"]))
    ps.ctxu_pool = ctx.enter_context(tc.tile_pool(name="ctxu", bufs=4))
    ps.linv_pool = ctx.enter_context(tc.tile_pool(name="linv", bufs=3))
    ps.ob_pool = ctx.enter_context(tc.tile_pool(name="ob", bufs=3))
    return ps


def _iteration(tc, io, ps):
    nc = tc.nc
    qt8_q, kt8_q, v_t, ctxn_q = ps.qt8_q, ps.kt8_q, ps.v_t, ps.ctxn_q
    xt_sb = ps.in_pool.tile([P, ET, S], BF, tag="xt", name="xt")
    xt8_sb = ps.in_pool.tile([P, ET, S], FP8, tag="xt8", name="xt8")
    wq8_sb = ps.in_pool.tile([P, ET, DH], FP8, tag="wq8", name="wq8")
    wk8_sb = ps.in_pool.tile([P, ET, DH], FP8, tag="wk8", name="wk8")
    wv_sb = ps.in_pool.tile([P, ET, DH], BF, tag="wv", name="wv")
    wo_sb = ps.in_pool.tile([P, DH // P, E], BF, tag="wo", name="wo")

    # qc-chunked input fill: fp8 Q/K weights + fp8 x chunks first (the Q/K
    # projections), then the bf16 x chunks for V, wv/wo slotted between.
    nc.sync.dma_start(wq8_sb[:], io.wq8)
    nc.sync.dma_start(
        xt8_sb[:, :, 0:QC], io.xt8[:, :, 0:QC],
    )
    nc.sync.dma_start(wk8_sb[:], io.wk8)
    for qc in range(NQC):
        if qc > 0:
            nc.sync.dma_start(
                xt8_sb[:, :, qc * QC:(qc + 1) * QC],
                io.xt8[:, :, qc * QC:(qc + 1) * QC],
            )
        nc.sync.dma_start(
            xt_sb[:, :, qc * QC:(qc + 1) * QC],
            io.xt[:, :, qc * QC:(qc + 1) * QC],
        )
        if qc == 0:
            nc.sync.dma_start(wv_sb[:], io.wv)
        elif qc == 1:
            nc.sync.dma_start(wo_sb[:], io.wo)

    # ---- fused projection + attention stream ------------------------------
    # PE is in-order, so the emission order is the schedule:
    #  - Q/K/V projections for chunk qc+1 and out-proj for chunk qc-1 are
    #    laced one item per unit into chunk qc's attention stream (sharing
    #    the po PSUM ring), so the first exp fires ~6us in instead of after
    #    the whole projection phase, and projection/out-proj matmuls soak
    #    up the PE slack while ACT works through the exps.
    #  - PV for unit ki is emitted pv_depth units late so its exp (ACT) and
    #    diag select (Pool) have score-matmul time to finish.
    #  - pvs PSUM banks are freed right after the last PV by ctxu copies;
    #    the reciprocal/broadcast/multiply chain is deferred into the next
    #    head-pair's first units.
    with tc.tile_pool(name="pv", bufs=1, space="PSUM") as pvp, \
         tc.tile_pool(name="st", bufs=OPTS["st_bufs"], space="PSUM") as stp, \
         tc.tile_pool(name="po", bufs=OPTS["po_bufs"], space="PSUM") as pop:

        def emit_qk_proj(qcc, w_sb, is_q, dt):
            # QT/KT [d, q] -> fp8 slot-0 tiles; fp8 DoubleRow over et pairs
            psum = pop.tile([P, QC], F32, tag="po", name="pj")
            for j in range(ET // 2):
                nc.tensor.matmul(
                    psum[:],
                    lhsT=w_sb[:, 2 * j:2 * j + 2, dt * P:(dt + 1) * P],
                    rhs=xt8_sb[:, 2 * j:2 * j + 2,
                               qcc * QC:(qcc + 1) * QC],
                    start=(j == 0), stop=(j == ET // 2 - 1),
                    perf_mode=DR,
                )
            dst = qt8_q[qcc] if is_q else kt8_q[qcc]
            nc.vector.tensor_copy(out=dst[:, dt, 0, :], in_=psum[:])

        def emit_v_proj(st):
            # V [s, d] bf16, per-head 66-wide slots in v_t
            psum = pop.tile([P, QC], F32, tag="po", name="pjv")
            for et in range(ET):
                nc.tensor.matmul(
                    psum[:, 0:DH],
                    lhsT=xt_sb[:, et, st * P:(st + 1) * P],
                    rhs=wv_sb[:, et, :],
                    start=(et == 0), stop=(et == ET - 1),
                )
            nc.vector.tensor_copy(
                out=v_t[st][:, :, 0:64],
                in_=psum[:, 0:DH].rearrange("p (h d) -> p h d", h=HPC),
            )

        ob_cur = [None]

        def emit_oproj(qcc, et, borrow=False):
            if borrow and et % 3:
                psum = pvp.tile([P, QC], F32, tag=f"pv{et % 2}", name="povv")
            else:
                psum = pop.tile([P, QC], F32, tag="po", name="po")
            for cc2 in range(DH // P):
                nc.tensor.matmul(
                    psum[:],
                    lhsT=wo_sb[:, cc2, et * P:(et + 1) * P],
                    rhs=ctxn_q[qcc][:, cc2, :],
                    start=(cc2 == 0), stop=(cc2 == DH // P - 1),
                )
            # bf16 partials, paired into one DMA per two e-tiles
            if ob_cur[0] is None:
                ob_cur[0] = ps.ob_pool.tile([P, 2, QC], BF, tag="ob",
                                            name="ob")
            ob = ob_cur[0]
            nc.vector.tensor_copy(out=ob[:, et % 2, :], in_=psum[:])
            if et % 2:
                nc.sync.dma_start(
                    io.outp.rearrange("(eo p) q -> p eo q", p=P)[
                        :, et - 1:et + 1, qcc * QC:(qcc + 1) * QC
                    ],
                    ob[:],
                )
                ob_cur[0] = None

        def proj_items(qcc):
            items = []
            for w_sb, is_q in ((wq8_sb, True), (wk8_sb, False)):
                for dt in range(NDT):
                    items.append(lambda w=w_sb, q=is_q, d=dt:
                                 emit_qk_proj(qcc, w, q, d))
            for st in range(qcc * NM, (qcc + 1) * NM):
                items.append(lambda s=st: emit_v_proj(s))
            return items

        # chunk 0 projections lead the stream
        for it in proj_items(0):
            it()

        norm_jobs = []

        def flush_norm():
            while norm_jobs:
                norm_jobs.pop(0)()

        for qc in range(NQC):
            nk = (qc + 1) * NM  # causal k-tiles for this chunk
            lace = proj_items(qc + 1) if qc + 1 < NQC else []
            if qc > 0:
                lace += [lambda e=et: emit_oproj(qc - 1, e)
                         for et in range(ET)]
            spread = max(1, (2 * nk) // (len(lace) + 1)) if lace else 0
            ucount = 0
            for hp in range(HPC // 2):
                cc = hp
                pvs = [pvp.tile([P, QC], F32, tag=f"pv{i}", name=f"pv{i}")
                       for i in range(2)]
                pending = []

                def emit_pv(ent):
                    ki2, stx2, off2 = ent
                    for i in range(2):
                        h = 2 * hp + i
                        nc.tensor.matmul(
                            pvs[i][0:65, off2:],
                            lhsT=v_t[ki2][:, h, 0:65],
                            rhs=stx2[:, i, off2:],
                            start=(ki2 == 0), stop=(ki2 == nk - 1),
                        )

                for ki in range(nk):
                    diag = ki >= qc * NM
                    m = ki - qc * NM if diag else 0
                    off = P * m if (diag and OPTS["narrow"]) else 0
                    # both heads' scores into one 2-bank PSUM tile via
                    # fp8 DoubleRow matmuls (slot 1 of qt8/kt8 is zero)
                    st_ps = stp.tile([P, 2, QC], F32, tag="st", name="st")
                    for i in range(2):
                        pr = 64 * i
                        nc.tensor.matmul(
                            st_ps[:, i, off:],
                            lhsT=kt8_q[ki // NM][pr:pr + 64, cc, :,
                                                 (ki % NM) * P:
                                                 (ki % NM + 1) * P],
                            rhs=qt8_q[qc][pr:pr + 64, cc, :, off:],
                            start=True, stop=True,
                            perf_mode=DR,
                        )
                    # one exp covers both heads (PSUM -> SBUF bf16)
                    stx = ps.stx_pool.tile([P, 2, QC], BF, tag="stx",
                                           name="stx")
                    nc.scalar.activation(
                        out=stx[:, :, off:], in_=st_ps[:, :, off:], func=Exp,
                        scale=float(SCALE / 1024.0),
                    )
                    if diag:
                        # zero the causally-invalid lower triangle of the
                        # diagonal 128-wide strip (Pool engine)
                        for i in range(2):
                            nc.gpsimd.affine_select(
                                out=stx[:, i, off:off + P],
                                in_=stx[:, i, off:off + P],
                                compare_op=mybir.AluOpType.is_ge, fill=0.0,
                                base=0, pattern=[[1, P]],
                                channel_multiplier=-1,
                            )
                    pending.append((ki, stx, off))
                    if ki == 1:
                        flush_norm()
                    if len(pending) > OPTS["pv_depth"]:
                        emit_pv(pending.pop(0))
                    ucount += 1
                    if lace and spread and ucount % spread == 0:
                        lace.pop(0)()
                while pending:
                    emit_pv(pending.pop(0))

                # normalize: free pv banks via ctxu copies, broadcast 1/sums
                # across partitions with a K=1 f32r matmul, multiply on DVE.
                # Deferred into the next head-pair's first units so the PE
                # broadcast matmul never stalls the stream.
                def norm_job(pvs=pvs, cc=cc, qc=qc):
                    ctxus = []
                    linv = ps.linv_pool.tile([1, 2, QC], mybir.dt.float32r,
                                             tag="linv", name="linv")
                    for i in range(2):
                        ctxu = ps.ctxu_pool.tile([65, QC], F32, tag="ctxu",
                                                 name="ctxu")
                        nc.vector.tensor_copy(out=ctxu[:], in_=pvs[i][0:65, :])
                        with nc.allow_low_precision(
                                reason="1/sums broadcast via f32r matmul; "
                                       "tf32 mantissa is plenty for the "
                                       "normalizer"):
                            nc.vector.reciprocal(linv[:, i, :],
                                                 ctxu[64:65, :])
                        ctxus.append(ctxu)
                    for i in range(2):
                        bc_ps = pop.tile([P, QC], F32, tag="po", name="bcps")
                        nc.tensor.matmul(
                            bc_ps[0:64, :],
                            lhsT=ps.ones_col[:],
                            rhs=linv[:, i, :],
                            start=True, stop=True,
                        )
                        nc.vector.tensor_tensor(
                            ctxn_q[qc][64 * i:64 * i + 64, cc, :],
                            ctxus[i][0:64, :], bc_ps[0:64, :], MULT,
                        )
                norm_jobs.append(norm_job)
            flush_norm()
            while lace:
                lace.pop(0)()
        # last chunk's out-proj: the pv banks are free afterwards — borrow
        # them so the tail pipelines deeper than the po ring alone.
        for et in range(ET):
            emit_oproj(NQC - 1, et, borrow=True)


_NC_CACHE = {}


def build_nc(iters=1):
    if iters not in _NC_CACHE:
        from contextlib import ExitStack
        nc = bass.Bass("TRN2", target_bir_lowering=False, debug=False)
        with TileContext(nc) as tc, ExitStack() as es:
            io = _declare_io(nc)
            ps = _persistent(es, tc)
            for _ in range(iters):
                _iteration(tc, io, ps)
        split_excess_waits(nc)
        _NC_CACHE[iters] = nc
    return _NC_CACHE[iters]


def make_in_maps(embeddings, wq, wk, wv, wo):
    bf = ml_dtypes.bfloat16
    f8 = ml_dtypes.float8_e4m3
    in_maps = []
    for c in range(NCORES):
        b, g = c // TP, c % TP
        cols = slice(g * DH, (g + 1) * DH)
        def _arr(a, dt=bf):  # [(c p), f] -> [p, c, f] contiguous
            c = a.shape[0] // 128
            return np.ascontiguousarray(
                a.reshape(c, 128, a.shape[1]).transpose(1, 0, 2)).astype(dt)
        xt = embeddings[b].T
        in_maps.append({
            "xt": _arr(xt),
            "xt8": _arr(xt, f8),
            "wq8": _arr(32.0 * wq[:, cols], f8),
            "wk8": _arr(32.0 * wk[:, cols], f8),
            "wv": _arr(wv[:, cols]),
            "wo": _arr(wo[cols, :]),
        })
    return in_maps


def assemble(results, bo):
    out = np.zeros((B, S, E), dtype=np.float32)
    for c in range(NCORES):
        b = c // TP
        out[b] += results[c]["outp"].T
    out += bo.astype(np.float32)
    return out


def kernel(embeddings, wq, wk, wv, wo, bo):
    embeddings = np.asarray(embeddings)
    nc = build_nc()
    in_maps = make_in_maps(embeddings, np.asarray(wq), np.asarray(wk),
                           np.asarray(wv), np.asarray(wo))
    res = run_bass_kernel_spmd(nc, in_maps, core_ids=list(range(NCORES)),
                               trace=False)
    return assemble(res.results, np.asarray(bo))


# revision 42
# speedup vs baseline: 1.3563x; 1.3563x over previous
"""Multi-head causal attention (B=2, S=2048, E=1024, H=16, D=64) on 8 TRN2
NeuronCores.

Sharding: 4-way tensor-parallel over heads x 2-way data-parallel over batch.
Core c handles batch b = c//4 and head group g = c%4 (heads 4g..4g+3).

Per-core device program:
  - Q/K projections run entirely in fp8e4 with MatmulPerfMode.DoubleRow
    (two 128-deep contraction tiles per instruction): lhsT = fp8(32*Wq/Wk)
    et-pair slices, rhs = fp8(X^T) et-pair slices, fp32 PSUM. The 32x
    weight prescale keeps W (std 0.02) out of the fp8 subnormal range; the
    resulting 1024x score scale is folded into the exp.
  - V = X @ Wv stays bf16 (fp8 there pushes max-rel-err past the 2e-2
    gate); a ones column is appended per head (V_aug) so the softmax
    denominators fall out of the PV matmul for free.
  - Q/K PSUM results are stored as fp8e4 tiles with a zero second
    DoubleRow slot, so the score matmuls also run DoubleRow at half cost:
    ST[k, q] = lhsT kt8[64, 2, 128] x rhs qt8[64, 2, q]. The two heads of
    a pair write one [P, 2, QC] two-bank PSUM tile and a single wide exp
    (ScalarE, PSUM -> SBUF bf16, scale = 1/(8*1024)) covers both heads.
  - Causal masking is post-exp: Pool-engine affine_select zeroes the lower
    triangle of the diagonal 128-wide strip of stx (no PE mask matmuls);
    fully-masked columns are skipped via the narrow-diag trick.
  - ctxT[d, q] (+ sums row) = V_aug^T @ P^T accumulated over k tiles in
    PSUM (bf16). PE is in-order, so the emission order is the schedule:
    there is no separate projection phase — Q/K/V projection groups for
    chunk qc+1 and out-proj matmuls for chunk qc-1 are laced one item per
    unit into chunk qc's attention stream (sharing the po PSUM ring), so
    the first exp fires ~6-9us in and projection work soaks up PE slack
    while ScalarE grinds through the exps (the ACT engine is the pacing
    engine for the attention phase). PV for unit ki is emitted pv_depth
    units late so its exp/select have score-matmul time to finish.
  - Normalize: ctx (+sums) -> SBUF f32 copies free the PV banks at once;
    1/sums (f32r) is broadcast across partitions by a K=1 f32r matmul
    (no DMA round-trip) and multiplied on DVE -> ctxn bf16. The whole
    chain is deferred into the next head-pair's first units so it never
    stalls the PE stream.
  - outT partial [e, q] = Wo_shard^T @ ctxn per q chunk; bf16 partials,
    two e-tiles per output DMA.
  - Input DMAs are fused per qc chunk (one descriptor batch per chunk)
    and double-buffered, so a chained iteration's fill overlaps the
    previous iteration's compute and the first projection starts ~3us in.

Host side: gather 8 partial outT tensors, out[b] = sum_g partial^T + bo.
"""

import numpy as np
import ml_dtypes

import bass_rust
import concourse.bass as bass
import concourse.tile as tile
from concourse import mybir
from concourse.tile import TileContext
from concourse.bass_utils import run_bass_kernel_spmd
from concourse.vector_clock import ScopedClock

# ---------------------------------------------------------------------------
# Workaround: this environment's walrus build caps the number of sync-wait
# commands encodable on a single instruction. Redistribute excess waits onto
# single-wait NOPs inserted immediately before the instruction on the same
# engine queue (program order preserves the blocking semantics).
# ---------------------------------------------------------------------------


def _patched_drain_and_barrier(self, tick_clock, wait_clock):
    nop0 = self.nc.sync.nop(nofuse=True)
    wait_clock.add_sem_waits(nop0.ins, ScopedClock({None: tick_clock.global_clock}))
    si = nop0.ins.sync_info
    if si is not None and si.on_wait and len(si.on_wait) > 1:
        waits = list(si.on_wait)
        nop0.ins.sync_info = bass_rust.SyncInfo(
            on_wait=[waits[0]], on_update=list(si.on_update or [])
        )
        for w in waits[1:]:
            n = self.nc.sync.nop(nofuse=True)
            n.ins.sync_info = bass_rust.SyncInfo(on_wait=[w], on_update=[])
    self.nc.sync.drain()
    self.nc.all_engine_barrier()
    assert self.sems is not None
    popped = self.nc._tile_sem_poison_stack.pop()
    assert popped is self._sem_poison
    self.nc.clear_and_free_semaphores(list(self.sems.allocated().values()))
    self.nc.all_engine_barrier()


tile.TileContext._drain_and_barrier = _patched_drain_and_barrier

MAX_WAITS = 1


def split_excess_waits(nc, max_waits=None):
    if max_waits is None:
        max_waits = MAX_WAITS
    for f in nc.m.functions:
        for bb in f.blocks:
            insts = bb.instructions
            out = []
            changed = False
            for inst in insts:
                si = inst.sync_info
                waits = list(si.on_wait) if si is not None and si.on_wait else []
                if len(waits) > max_waits:
                    changed = True
                    excess, keep = waits[:-max_waits], waits[-max_waits:]
                    for w in excess:
                        out.append(mybir.InstNoOp(
                            name=nc.get_next_instruction_name(),
                            engine=inst.engine,
                            bass_nofuse=True,
                            sync_info=mybir.SyncInfo(on_wait=[w], on_update=[]),
                        ))
                    inst.sync_info = mybir.SyncInfo(
                        on_wait=keep, on_update=list(si.on_update or [])
                    )
                out.append(inst)
            if changed:
                bb.instructions = out


# ---------------------------------------------------------------------------
# Problem constants (hardcoded per the harness contract).
# ---------------------------------------------------------------------------

B, S, E = 2, 2048, 1024
H, D = 16, 64
NCORES, TP = 8, 4
HPC = H // TP          # heads per core = 4
DH = HPC * D           # head-dim columns per core = 256
P = 128                # partitions
QC = 512               # q chunk (free dim of score matmuls)
NQC = S // QC          # 4 q chunks
NKT = S // P           # 16 k tiles
SCALE = 1.0 / np.sqrt(np.float32(D))

OPTS = {
    "st_bufs": 2,
    "stx_bufs": 7,
    "po_bufs": 2,
    "pv_depth": 4,
    "narrow": True,
}

BF = mybir.dt.bfloat16
F32 = mybir.dt.float32
FP8 = mybir.dt.float8e4
Exp = mybir.ActivationFunctionType.Exp
MULT = mybir.AluOpType.mult
DR = mybir.MatmulPerfMode.DoubleRow


class _Persist:
    pass


def _declare_io(nc):
    io = _Persist()
    # inputs are pre-arranged on host to [partition, chunk, free] so every
    # DMA is contiguous per partition
    io.xt = nc.dram_tensor("xt", [P, E // P, S], BF, kind="ExternalInput").ap()
    io.xt8 = nc.dram_tensor("xt8", [P, E // P, S], FP8, kind="ExternalInput").ap()
    io.wq8 = nc.dram_tensor("wq8", [P, E // P, DH], FP8, kind="ExternalInput").ap()
    io.wk8 = nc.dram_tensor("wk8", [P, E // P, DH], FP8, kind="ExternalInput").ap()
    io.wv = nc.dram_tensor("wv", [P, E // P, DH], BF, kind="ExternalInput").ap()
    io.wo = nc.dram_tensor("wo", [P, DH // P, E], BF, kind="ExternalInput").ap()
    io.outp = nc.dram_tensor("outp", [E, S], BF, kind="ExternalOutput").ap()
    return io


ET = E // P   # 8 e tiles
NM = QC // P  # 4 k-tiles per q chunk
NDT = DH // P  # 2 head-pair column blocks


def _persistent(ctx, tc):
    nc = tc.nc
    ps = _Persist()
    consts = ctx.enter_context(tc.tile_pool(name="consts", bufs=1))

    # xt + weights are double-buffered so a chained iteration's input fill
    # overlaps the previous iteration's compute
    ps.in_pool = ctx.enter_context(tc.tile_pool(name="inp", bufs=2))
    # fp8 Q/K with a zero second DoubleRow slot:
    # qt8[qc]: [P, pair, slot, q]; kt8[ki]: [P, pair, slot, k]
    ps.qt8_q = [consts.tile([P, NDT, 2, QC], FP8, tag=f"qt{qc}",
                            name=f"qt{qc}") for qc in range(NQC)]
    ps.kt8_q = [consts.tile([P, NDT, 2, QC], FP8, tag=f"kt{kq}", name=f"kt{kq}")
                for kq in range(NQC)]
    ps.v_t = [consts.tile([P, HPC, 66], BF, tag=f"v{ki}", name=f"v{ki}")
              for ki in range(NKT)]
    ps.ctxn_q = [consts.tile([P, DH // P, QC], BF, tag=f"ctxn{qc}",
                             name=f"ctxn{qc}") for qc in range(NQC)]

    # f32r ones column for the linv PE-broadcast matmul (memset cannot
    # target f32r, so fill f32 scratch and convert with a DVE copy)
    ps.ones_col = consts.tile([1, 64], mybir.dt.float32r, tag="ones64")
    ones_f = consts.tile([1, 64], F32, tag="ones64f")
    nc.gpsimd.memset(ones_f[:], 1.0)
    with nc.allow_low_precision(reason="f32->f32r ones for linv bcast"):
        nc.vector.tensor_copy(out=ps.ones_col[:], in_=ones_f[:])

    # zero DoubleRow slots (written once; iterations only touch slot 0)
    for qc in range(NQC):
        nc.gpsimd.memset(ps.qt8_q[qc][:, :, 1, :], 0.0)
    for kq in range(NQC):
        nc.gpsimd.memset(ps.kt8_q[kq][:, :, 1, :], 0.0)
    for ki in range(NKT):
        # ones column for V_aug
        nc.gpsimd.memset(ps.v_t[ki][:, :, 64:66], 0.0)
        nc.gpsimd.memset(ps.v_t[ki][:, :, 64:65], 1.0)

    ps.stx_pool = ctx.enter_context(tc.tile_pool(name="stx", bufs=OPTS["stx_bufs"]))
    ps.ctxu_pool = ctx.enter_context(tc.tile_pool(name="ctxu", bufs=4))
    ps.linv_pool = ctx.enter_context(tc.tile_pool(name="linv", bufs=3))
    ps.ob_pool = ctx.enter_context(tc.tile_pool(name="ob", bufs=3))
    return ps


def _iteration(tc, io, ps):
    nc = tc.nc
    qt8_q, kt8_q, v_t, ctxn_q = ps.qt8_q, ps.kt8_q, ps.v_t, ps.ctxn_q
    xt_sb = ps.in_pool.tile([P, ET, S], BF, tag="xt", name="xt")
    xt8_sb = ps.in_pool.tile([P, ET, S], FP8, tag="xt8", name="xt8")
    wq8_sb = ps.in_pool.tile([P, ET, DH], FP8, tag="wq8", name="wq8")
    wk8_sb = ps.in_pool.tile([P, ET, DH], FP8, tag="wk8", name="wk8")
    wv_sb = ps.in_pool.tile([P, ET, DH], BF, tag="wv", name="wv")
    wo_sb = ps.in_pool.tile([P, DH // P, E], BF, tag="wo", name="wo")

    # qc-chunked input fill: fp8 Q/K weights + fp8 x chunks first (the Q/K
    # projections), then the bf16 x chunks for V, wv/wo slotted between.
    nc.sync.dma_start(wq8_sb[:], io.wq8)
    nc.sync.dma_start(
        xt8_sb[:, :, 0:QC], io.xt8[:, :, 0:QC],
    )
    nc.sync.dma_start(wk8_sb[:], io.wk8)
    for qc in range(NQC):
        if qc > 0:
            nc.sync.dma_start(
                xt8_sb[:, :, qc * QC:(qc + 1) * QC],
                io.xt8[:, :, qc * QC:(qc + 1) * QC],
            )
        nc.sync.dma_start(
            xt_sb[:, :, qc * QC:(qc + 1) * QC],
            io.xt[:, :, qc * QC:(qc + 1) * QC],
        )
        if qc == 0:
            nc.sync.dma_start(wv_sb[:], io.wv)
        elif qc == 1:
            nc.sync.dma_start(wo_sb[:], io.wo)

    # ---- fused projection + attention stream ------------------------------
    # PE is in-order, so the emission order is the schedule:
    #  - Q/K/V projections for chunk qc+1 and out-proj for chunk qc-1 are
    #    laced one item per unit into chunk qc's attention stream (sharing
    #    the po PSUM ring), so the first exp fires ~6us in instead of after
    #    the whole projection phase, and projection/out-proj matmuls soak
    #    up the PE slack while ACT works through the exps.
    #  - PV for unit ki is emitted pv_depth units late so its exp (ACT) and
    #    diag select (Pool) have score-matmul time to finish.
    #  - pvs PSUM banks are freed right after the last PV by ctxu copies;
    #    the reciprocal/broadcast/multiply chain is deferred into the next
    #    head-pair's first units.
    with tc.tile_pool(name="pv", bufs=1, space="PSUM") as pvp, \
         tc.tile_pool(name="st", bufs=OPTS["st_bufs"], space="PSUM") as stp, \
         tc.tile_pool(name="po", bufs=OPTS["po_bufs"], space="PSUM") as pop:

        def emit_qk_proj(qcc, w_sb, is_q, dt):
            # QT/KT [d, q] -> fp8 slot-0 tiles; fp8 DoubleRow over et pairs
            psum = pop.tile([P, QC], F32, tag="po", name="pj")
            for j in range(ET // 2):
                nc.tensor.matmul(
                    psum[:],
                    lhsT=w_sb[:, 2 * j:2 * j + 2, dt * P:(dt + 1) * P],
                    rhs=xt8_sb[:, 2 * j:2 * j + 2,
                               qcc * QC:(qcc + 1) * QC],
                    start=(j == 0), stop=(j == ET // 2 - 1),
                    perf_mode=DR,
                )
            dst = qt8_q[qcc] if is_q else kt8_q[qcc]
            nc.vector.tensor_copy(out=dst[:, dt, 0, :], in_=psum[:])

        def emit_v_proj(st):
            # V [s, d] bf16, per-head 66-wide slots in v_t
            psum = pop.tile([P, QC], F32, tag="po", name="pjv")
            for et in range(ET):
                nc.tensor.matmul(
                    psum[:, 0:DH],
                    lhsT=xt_sb[:, et, st * P:(st + 1) * P],
                    rhs=wv_sb[:, et, :],
                    start=(et == 0), stop=(et == ET - 1),
                )
            nc.vector.tensor_copy(
                out=v_t[st][:, :, 0:64],
                in_=psum[:, 0:DH].rearrange("p (h d) -> p h d", h=HPC),
            )

        ob_cur = [None]

        def emit_oproj(qcc, et, borrow=False):
            if borrow and et % 3:
                psum = pvp.tile([P, QC], F32, tag=f"pv{et % 2}", name="povv")
            else:
                psum = pop.tile([P, QC], F32, tag="po", name="po")
            for cc2 in range(DH // P):
                nc.tensor.matmul(
                    psum[:],
                    lhsT=wo_sb[:, cc2, et * P:(et + 1) * P],
                    rhs=ctxn_q[qcc][:, cc2, :],
                    start=(cc2 == 0), stop=(cc2 == DH // P - 1),
                )
            # bf16 partials, paired into one DMA per two e-tiles
            if ob_cur[0] is None:
                ob_cur[0] = ps.ob_pool.tile([P, 2, QC], BF, tag="ob",
                                            name="ob")
            ob = ob_cur[0]
            nc.vector.tensor_copy(out=ob[:, et % 2, :], in_=psum[:])
            if et % 2:
                nc.sync.dma_start(
                    io.outp.rearrange("(eo p) q -> p eo q", p=P)[
                        :, et - 1:et + 1, qcc * QC:(qcc + 1) * QC
                    ],
                    ob[:],
                )
                ob_cur[0] = None

        def proj_items(qcc):
            items = []
            for w_sb, is_q in ((wq8_sb, True), (wk8_sb, False)):
                for dt in range(NDT):
                    items.append(lambda w=w_sb, q=is_q, d=dt:
                                 emit_qk_proj(qcc, w, q, d))
            for st in range(qcc * NM, (qcc + 1) * NM):
                items.append(lambda s=st: emit_v_proj(s))
            return items

        # chunk 0 projections lead the stream
        for it in proj_items(0):
            it()

        queue = []
        delayed = []

        def unit_tick():
            for j in range(len(delayed) - 1, -1, -1):
                c, fn = delayed[j]
                if c <= 1:
                    queue.append(fn)
                    delayed.pop(j)
                else:
                    delayed[j] = (c - 1, fn)
            pops = 0
            while queue and len(queue) > OPTS["pv_depth"] and pops < 3:
                queue.pop(0)()
                pops += 1

        norm_done = [0] * NQC

        def force_pop():
            if queue:
                queue.pop(0)()
                return True
            if delayed:
                for c, fn in delayed:
                    queue.append(fn)
                delayed.clear()
                queue.pop(0)()
                return True
            return False

        def drain_all():
            while queue or delayed:
                for c, fn in delayed:
                    queue.append(fn)
                delayed.clear()
                while queue:
                    queue.pop(0)()

        for qc in range(NQC):
            nk = (qc + 1) * NM
            lace = proj_items(qc + 1) if qc + 1 < NQC else []
            if qc > 0:
                def oproj_guarded(e, qcc=qc - 1):
                    # ctxn[qcc] must be fully written (both head-pairs'
                    # deferred normalize part-Bs emitted) before any
                    # out-proj matmul reads it
                    while norm_done[qcc] < 2:
                        if not force_pop():
                            break
                    emit_oproj(qcc, e)
                lace += [lambda e=et: oproj_guarded(e) for et in range(ET)]
            spread = max(1, (2 * nk) // (len(lace) + 1)) if lace else 0
            ucount = 0
            for hp in range(HPC // 2):
                cc = hp
                pvs = []

                def emit_pv(ki2, stx2, off2, pvs=pvs, hp=hp, nk=nk):
                    if not pvs:
                        pvs.extend(pvp.tile([P, QC], F32, tag=f"pv{i}",
                                            name=f"pv{i}")
                                   for i in range(2))
                    for i in range(2):
                        h = 2 * hp + i
                        nc.tensor.matmul(
                            pvs[i][0:65, off2:],
                            lhsT=v_t[ki2][:, h, 0:65],
                            rhs=stx2[:, i, off2:],
                            start=(ki2 == 0), stop=(ki2 == nk - 1),
                        )

                for ki in range(nk):
                    diag = ki >= qc * NM
                    m = ki - qc * NM if diag else 0
                    off = P * m if (diag and OPTS["narrow"]) else 0
                    st_ps = stp.tile([P, 2, QC], F32, tag="st", name="st")
                    for i in range(2):
                        pr = 64 * i
                        nc.tensor.matmul(
                            st_ps[:, i, off:],
                            lhsT=kt8_q[ki // NM][pr:pr + 64, cc, :,
                                                 (ki % NM) * P:
                                                 (ki % NM + 1) * P],
                            rhs=qt8_q[qc][pr:pr + 64, cc, :, off:],
                            start=True, stop=True,
                            perf_mode=DR,
                        )
                    stx = ps.stx_pool.tile([P, 2, QC], BF, tag="stx",
                                           name="stx")
                    nc.scalar.activation(
                        out=stx[:, :, off:], in_=st_ps[:, :, off:], func=Exp,
                        scale=float(SCALE / 1024.0),
                    )
                    if diag:
                        for i in range(2):
                            nc.gpsimd.affine_select(
                                out=stx[:, i, off:off + P],
                                in_=stx[:, i, off:off + P],
                                compare_op=mybir.AluOpType.is_ge, fill=0.0,
                                base=0, pattern=[[1, P]],
                                channel_multiplier=-1,
                            )
                    queue.append(lambda k=ki, s=stx, o=off, f=emit_pv:
                                 f(k, s, o))
                    unit_tick()
                    ucount += 1
                    if lace and spread and ucount % spread == 0:
                        lace.pop(0)()

                def norm_part_a(pvs=pvs, cc=cc, qc=qc):
                    ctxus = []
                    linv = ps.linv_pool.tile([1, 2, QC], mybir.dt.float32r,
                                             tag="linv", name="linv")
                    for i in range(2):
                        ctxu = ps.ctxu_pool.tile([65, QC], F32, tag="ctxu",
                                                 name="ctxu")
                        nc.vector.tensor_copy(out=ctxu[:], in_=pvs[i][0:65, :])
                        with nc.allow_low_precision(
                                reason="f32r linv"):
                            nc.vector.reciprocal(linv[:, i, :],
                                                 ctxu[64:65, :])
                        ctxus.append(ctxu)

                    def norm_part_b():
                        for i in range(2):
                            bc_ps = pop.tile([P, QC], F32, tag="po",
                                             name="bcps")
                            nc.tensor.matmul(
                                bc_ps[0:64, :],
                                lhsT=ps.ones_col[:],
                                rhs=linv[:, i, :],
                                start=True, stop=True,
                            )
                            nc.vector.tensor_tensor(
                                ctxn_q[qc][64 * i:64 * i + 64, cc, :],
                                ctxus[i][0:64, :], bc_ps[0:64, :], MULT,
                            )
                        norm_done[qc] += 1
                    delayed.append((2, norm_part_b))
                queue.append(norm_part_a)
            while lace:
                lace.pop(0)()
        drain_all()
        # last chunk's out-proj: the pv banks are free afterwards — borrow
        # them so the tail pipelines deeper than the po ring alone.
        for et in range(ET):
            emit_oproj(NQC - 1, et, borrow=True)


_NC_CACHE = {}


def build_nc(iters=1):
    if iters not in _NC_CACHE:
        from contextlib import ExitStack
        nc = bass.Bass("TRN2", target_bir_lowering=False, debug=False)
        with TileContext(nc) as tc, ExitStack() as es:
            io = _declare_io(nc)
            nc._io = io
            ps = _persistent(es, tc)
            for _ in range(iters):
                _iteration(tc, io, ps)
        split_excess_waits(nc)
        _NC_CACHE[iters] = nc
    return _NC_CACHE[iters]


def make_in_maps(embeddings, wq, wk, wv, wo):
    bf = ml_dtypes.bfloat16
    f8 = ml_dtypes.float8_e4m3
    in_maps = []
    for c in range(NCORES):
        b, g = c // TP, c % TP
        cols = slice(g * DH, (g + 1) * DH)
        def _arr(a, dt=bf):  # [(c p), f] -> [p, c, f] contiguous
            c = a.shape[0] // 128
            return np.ascontiguousarray(
                a.reshape(c, 128, a.shape[1]).transpose(1, 0, 2)).astype(dt)
        xt = embeddings[b].T
        in_maps.append({
            "xt": _arr(xt),
            "xt8": _arr(xt, f8),
            "wq8": _arr(32.0 * wq[:, cols], f8),
            "wk8": _arr(32.0 * wk[:, cols], f8),
            "wv": _arr(wv[:, cols]),
            "wo": _arr(wo[cols, :]),
        })
    return in_maps


def assemble(results, bo):
    out = np.zeros((B, S, E), dtype=np.float32)
    for c in range(NCORES):
        b = c // TP
        out[b] += results[c]["outp"].T
    out += bo.astype(np.float32)
    return out


def kernel(embeddings, wq, wk, wv, wo, bo):
    embeddings = np.asarray(embeddings)
    nc = build_nc()
    in_maps = make_in_maps(embeddings, np.asarray(wq), np.asarray(wk),
                           np.asarray(wv), np.asarray(wo))
    res = run_bass_kernel_spmd(nc, in_maps, core_ids=list(range(NCORES)),
                               trace=False)
    return assemble(res.results, np.asarray(bo))
